# revision 8
# baseline (speedup 1.0000x reference)
"""BitLinear (binarized linear + activation-LN) Trainium2 kernel.

Full-input contract: kernel(x[8192,2048] f32, weight[2048,2048] f32,
bias[2048] f32) -> y[8192,2048] f32, data-parallel over 8 NeuronCores
(1024 rows of x per core; weight/bias replicated).

Math (gama cancels exactly between the activation quant scale and the
output dequant scale, and the clip at +-(qb-eps) only perturbs the row
max by ~1e-6 relative):

    y[b,o] = beta_o * ( sum_i (r_b x[b,i]) sign(w[o,i]-mu_o)
                        - (r_b mu_b) S[o] + bias[o]/beta_o )

with S[o] = sum_i sign(w[o,i]-mu_o) (free from the ACT sign pass
accum_out), the (-r*mu, 1) x (S, bias/beta) terms folded in as a K=2
correction matmul that seeds each PSUM accumulation group, and the
beta_o column scale applied in the PSUM->SBUF epilogue.

Both matmul operands are transposed via single 3D-out DMA xbar
transposes, which land row i at (p=i//16, k=i%16); the contraction is
permutation-invariant so matching interleaves on both operands are
equivalent to blocked layouts.
"""

import os

import numpy as np

import concourse.bass as bass
import concourse.mybir as mybir
import concourse.tile as tile
from concourse import bacc
from concourse.bass_utils import run_bass_kernel_spmd

N_CORES = 8
BATCH = 8192
IN_F = 2048
OUT_F = 2048
B = BATCH // N_CORES  # rows of x per core
P = 128
KT = IN_F // P   # contraction blocks
OT = OUT_F // P  # weight row tiles
BT = B // P      # x row tiles per core
NC_CHUNK = 512   # matmul moving free dim
OC = OUT_F // NC_CHUNK
EPS = 1e-5

F16 = mybir.dt.float16
F32 = mybir.dt.float32
MUL = mybir.AluOpType.mult
AXF = mybir.AxisListType.X
AF = mybir.ActivationFunctionType


def _build_program() -> bass.Bass:
    nc = bacc.Bacc("TRN2", target_bir_lowering=False, debug=False)

    x16_h = nc.dram_tensor("x16", [B, IN_F], F16, kind="ExternalInput")
    w_h = nc.dram_tensor("w", [OUT_F, IN_F], F32, kind="ExternalInput")
    bias_h = nc.dram_tensor("bias16", [1, OUT_F], F16, kind="ExternalInput")
    y_h = nc.dram_tensor("y16", [B, OUT_F], F16, kind="ExternalOutput")
    # tiny DRAM bounces for partition->free row rearrangement
    nmr_h = nc.dram_tensor("nmr_d", [1, B], F16)
    r_h = nc.dram_tensor("r_d", [1, B], F16)
    s_h = nc.dram_tensor("s_d", [1, OUT_F], F16)
    beta_h = nc.dram_tensor("beta_d", [1, OUT_F], F16)
    bpr_h = nc.dram_tensor("bpr_d", [1, OUT_F], F16)

    x16 = x16_h[:, :]
    w = w_h[:, :]
    bias16 = bias_h[:, :]
    y16 = y_h[:, :]

    with tile.TileContext(nc) as tc:
        with (
            tc.tile_pool(name="consts", bufs=1) as consts,
            tc.tile_pool(name="persist", bufs=1) as persist,
            tc.tile_pool(name="wpool", bufs=3) as wpool,
            tc.tile_pool(name="wspool", bufs=3) as wspool,
            tc.tile_pool(name="xpool", bufs=3) as xpool,
            tc.tile_pool(name="stats", bufs=4) as stats,
            tc.tile_pool(name="trash", bufs=2) as trash,
            tc.tile_pool(name="ypool", bufs=3) as ypool,
            tc.tile_pool(name="psum", bufs=2, space="PSUM") as psum,
        ):
            eps_t = consts.tile([P, 1], F32)
            nc.vector.memset(eps_t, EPS)

            # persistent operands
            wsT = persist.tile([P, KT, OUT_F], F16)    # sign(w-mu)^T  [i, o]
            xT = persist.tile([P, KT, B], F16)         # x^T -> r*x^T in place
            corr_lhsT = persist.tile([2, B], F16)      # rows: -mu*r, ones
            corr_rhs = persist.tile([2, OUT_F], F16)   # rows: S, bias/beta
            r_bcast = persist.tile([P, B], F16)
            beta_bcast = persist.tile([P, OUT_F], F16)
            stats_cols = persist.tile([P, P], F16)     # cols 0..7 -mu*r, 8..15 r
            statsT = persist.tile([P, P], F16)
            # cols 0..15 S, 16..31 beta, 32..47 bias/beta per o-tile
            s_cols = persist.tile([P, P], F16)
            sT = persist.tile([P, P], F16)
            bias_cols = persist.tile([P, OT], F16)

            nc.vector.memset(stats_cols, 0.0)
            nc.vector.memset(s_cols, 0.0)
            # ones row for the corr matmul; engine ops can't target partition
            # 1 directly, so memset at partition 0 and DMA into place
            ones_row = consts.tile([1, B], F16)
            nc.vector.memset(ones_row, 1.0)
            nc.sync.dma_start(out=corr_lhsT[1:2, :], in_=ones_row)

            # bias in column layout [128, OT] (transposed load)
            bias_tiles = bias16.rearrange("a (t c) -> (a t) c", c=P)  # [16,128]
            nc.sync.dma_start(out=bias_cols, in_=bias_tiles, transpose=True)

            # ---------------- weight pipeline ----------------
            for ot in range(OT):
                wt = wpool.tile([P, IN_F], F32)
                nc.sync.dma_start(out=wt, in_=w[ot * P:(ot + 1) * P, :])

                sumw = stats.tile([P, 1], F32)
                nc.vector.reduce_sum(out=sumw, in_=wt, axis=AXF)
                negmean = stats.tile([P, 1], F32)
                nc.vector.tensor_scalar_mul(
                    out=negmean, in0=sumw, scalar1=-1.0 / IN_F)

                tr = trash.tile([P, IN_F], F16)
                asum = stats.tile([P, 1], F32)
                nc.scalar.activation(
                    out=tr, in_=wt, func=AF.Abs, accum_out=asum)
                beta = stats.tile([P, 1], F32)
                nc.vector.tensor_scalar_mul(
                    out=beta, in0=asum, scalar1=1.0 / IN_F)

                wb = wspool.tile([P, IN_F], F16)
                ssum = stats.tile([P, 1], F32)
                nc.scalar.activation(
                    out=wb, in_=wt, func=AF.Sign, bias=negmean, scale=1.0,
                    accum_out=ssum)

                # S, beta, bias/beta columns for this o-tile
                nc.vector.tensor_copy(out=s_cols[:, ot:ot + 1], in_=ssum)
                nc.vector.tensor_copy(
                    out=s_cols[:, 16 + ot:16 + ot + 1], in_=beta)
                binv = stats.tile([P, 1], F32)
                nc.vector.reciprocal(out=binv, in_=beta)
                nc.vector.tensor_mul(
                    out=s_cols[:, 32 + ot:32 + ot + 1],
                    in0=bias_cols[:, ot:ot + 1], in1=binv)

                # one 3D-out xbar transpose per o-tile: logical [2048, 128]
                # transpose lands row i at (p=i//KT, k=i%KT)
                nc.sync.dma_start(
                    out=wsT[:, :, ot * P:(ot + 1) * P], in_=wb,
                    transpose=True)

            # S/beta/bias' -> row layout [1, OUT_F] via xbar + DRAM bounce
            nc.sync.dma_start(out=sT, in_=s_cols, transpose=True)
            s_rows = s_h[0:1, :].rearrange("a (b c) -> (a b) c", c=P)
            beta_rows = beta_h[0:1, :].rearrange("a (b c) -> (a b) c", c=P)
            bpr_rows = bpr_h[0:1, :].rearrange("a (b c) -> (a b) c", c=P)
            nc.sync.dma_start(out=s_rows, in_=sT[0:OT, :])
            nc.sync.dma_start(out=beta_rows, in_=sT[16:16 + OT, :])
            nc.sync.dma_start(out=bpr_rows, in_=sT[32:32 + OT, :])
            nc.sync.dma_start(out=corr_rhs[0:1, :], in_=s_h[0:1, :])
            nc.sync.dma_start(out=corr_rhs[1:2, :], in_=bpr_h[0:1, :])
            b_src = beta_h[0:1, :]
            b_bc_ap = bass.AP(
                tensor=b_src.tensor, offset=b_src.offset,
                ap=[[0, P], [1, OUT_F]])
            nc.gpsimd.dma_start(out=beta_bcast, in_=b_bc_ap)

            # ---------------- x pipeline ----------------
            for bt in range(BT):
                xt = xpool.tile([P, IN_F], F16)
                nc.sync.dma_start(out=xt, in_=x16[bt * P:(bt + 1) * P, :])
                st = stats.tile([P, 4, 6], F32)
                for g in range(4):
                    nc.vector.bn_stats(
                        out=st[:, g, :], in_=xt[:, g * 512:(g + 1) * 512])
                mv = stats.tile([P, 2], F32)
                nc.vector.bn_aggr(out=mv, in_=st)
                std = stats.tile([P, 1], F32)
                nc.scalar.activation(
                    out=std, in_=mv[:, 1:2], func=AF.Sqrt, bias=eps_t,
                    scale=1.0)
                r32 = stats.tile([P, 1], F32)
                nc.vector.reciprocal(out=r32, in_=std)
                # -(mu * r) and r columns, fp16
                nc.vector.tensor_scalar(
                    out=stats_cols[:, bt:bt + 1], in0=mv[:, 0:1], scalar1=r32,
                    scalar2=-1.0, op0=MUL, op1=MUL)
                nc.vector.tensor_copy(
                    out=stats_cols[:, 8 + bt:8 + bt + 1], in_=r32)

            # transposed load of x (DRAM -> SBUF via xbar), same 3D-out row
            # interleave as wsT: row i -> (p=i//KT, k=i%KT)
            nc.sync.dma_start(out=xT, in_=x16, transpose=True)

            # stats -> row layout + broadcast
            nc.sync.dma_start(out=statsT, in_=stats_cols, transpose=True)
            nmr_rows = nmr_h[0:1, :].rearrange("a (b c) -> (a b) c", c=P)
            r_rows = r_h[0:1, :].rearrange("a (b c) -> (a b) c", c=P)
            nc.sync.dma_start(out=nmr_rows, in_=statsT[0:BT, :])
            nc.sync.dma_start(out=r_rows, in_=statsT[8:8 + BT, :])
            nc.sync.dma_start(out=corr_lhsT[0:1, :], in_=nmr_h[0:1, :])
            r_src = r_h[0:1, :]
            r_bc_ap = bass.AP(
                tensor=r_src.tensor, offset=r_src.offset,
                ap=[[0, P], [1, B]])
            nc.gpsimd.dma_start(out=r_bcast, in_=r_bc_ap)

            # scale x^T by r (in place)
            for k in range(KT):
                nc.vector.tensor_mul(
                    out=xT[:, k, :], in0=xT[:, k, :], in1=r_bcast)

            # ---------------- matmul + epilogue ----------------
            for bt in range(BT):
                ps = psum.tile([P, OUT_F], F32)
                bsl = slice(bt * P, (bt + 1) * P)
                for oc in range(OC):
                    osl = slice(oc * NC_CHUNK, (oc + 1) * NC_CHUNK)
                    nc.tensor.matmul(
                        ps[:, osl], corr_lhsT[:, bsl], corr_rhs[:, osl],
                        start=True, stop=False)
                for k in range(KT):
                    for oc in range(OC):
                        osl = slice(oc * NC_CHUNK, (oc + 1) * NC_CHUNK)
                        nc.tensor.matmul(
                            ps[:, osl], xT[:, k, bsl], wsT[:, k, osl],
                            start=False, stop=(k == KT - 1))
                ysb = ypool.tile([P, OUT_F], F16)
                nc.vector.tensor_mul(out=ysb, in0=ps, in1=beta_bcast)
                nc.sync.dma_start(out=y16[bt * P:(bt + 1) * P, :], in_=ysb)

    return nc


_NC_CACHE = None
LAST_RESULT = None


def _get_program():
    global _NC_CACHE
    if _NC_CACHE is None:
        nc = _build_program()
        nc.finalize()
        _NC_CACHE = nc
    return _NC_CACHE


def kernel(x: np.ndarray, weight: np.ndarray, bias: np.ndarray) -> np.ndarray:
    global LAST_RESULT
    assert x.shape == (BATCH, IN_F) and weight.shape == (OUT_F, IN_F)

    nc = _get_program()

    x16 = np.ascontiguousarray(x.astype(np.float16))
    w32 = np.ascontiguousarray(weight.astype(np.float32))
    b16 = np.ascontiguousarray(bias.astype(np.float16).reshape(1, OUT_F))

    in_maps = []
    for c in range(N_CORES):
        in_maps.append({
            "x16": np.ascontiguousarray(x16[c * B:(c + 1) * B, :]),
            "w": w32,
            "bias16": b16,
        })

    trace = bool(int(os.environ.get("BITLIN_TRACE", "0")))
    res = run_bass_kernel_spmd(
        nc, in_maps, core_ids=list(range(N_CORES)), trace=trace)
    LAST_RESULT = res

    y = np.concatenate(
        [np.asarray(res.results[c]["y16"]) for c in range(N_CORES)], axis=0)
    return y.astype(np.float32)


# revision 12
# speedup vs baseline: 1.0141x; 1.0141x over previous
"""BitLinear (binarized linear + activation-LN) Trainium2 kernel.

Full-input contract: kernel(x[8192,2048] f32, weight[2048,2048] f32,
bias[2048] f32) -> y[8192,2048] f32, data-parallel over 8 NeuronCores
(1024 rows of x per core; weight/bias replicated).

Math (gama cancels exactly between the activation quant scale and the
output dequant scale, and the clip at +-(qb-eps) only perturbs the row
max by ~1e-6 relative):

    y[b,o] = beta_o * ( sum_i (r_b x[b,i]) sign(w[o,i]-mu_o)
                        - (r_b mu_b) S[o] + bias[o]/beta_o )

with S[o] = sum_i sign(w[o,i]-mu_o) (free from the ACT sign pass
accum_out), the (-r*mu, 1) x (S, bias/beta) terms folded in as a K=2
correction matmul that seeds each PSUM accumulation group, and the
beta_o column scale applied in the PSUM->SBUF epilogue.

Both matmul operands are transposed via single 3D-out DMA xbar
transposes, which land row i at (p=i//16, k=i%16); the contraction is
permutation-invariant so matching interleaves on both operands are
equivalent to blocked layouts.
"""

import os

import numpy as np

import concourse.bass as bass
import concourse.mybir as mybir
import concourse.tile as tile
from concourse import bacc
from concourse.bass_utils import run_bass_kernel_spmd

N_CORES = 8
BATCH = 8192
IN_F = 2048
OUT_F = 2048
B = BATCH // N_CORES  # rows of x per core
P = 128
KT = IN_F // P   # contraction blocks
OT = OUT_F // P  # weight row tiles
BT = B // P      # x row tiles per core
NC_CHUNK = 512   # matmul moving free dim
OC = OUT_F // NC_CHUNK
EPS = 1e-5

F16 = mybir.dt.float16
F32 = mybir.dt.float32
MUL = mybir.AluOpType.mult
AXF = mybir.AxisListType.X
AF = mybir.ActivationFunctionType


def _build_program() -> bass.Bass:
    nc = bacc.Bacc("TRN2", target_bir_lowering=False, debug=False)

    x16_h = nc.dram_tensor("x16", [B, IN_F], F16, kind="ExternalInput")
    w_h = nc.dram_tensor("w", [OUT_F, IN_F], F32, kind="ExternalInput")
    bias_h = nc.dram_tensor("bias16", [1, OUT_F], F16, kind="ExternalInput")
    y_h = nc.dram_tensor("y16", [B, OUT_F], F16, kind="ExternalOutput")
    # tiny DRAM bounces for partition->free row rearrangement
    nmr_h = nc.dram_tensor("nmr_d", [1, B], F16)
    r_h = nc.dram_tensor("r_d", [1, B], F16)
    s_h = nc.dram_tensor("s_d", [1, OUT_F], F16)
    beta_h = nc.dram_tensor("beta_d", [1, OUT_F], F16)
    bpr_h = nc.dram_tensor("bpr_d", [1, OUT_F], F16)

    x16 = x16_h[:, :]
    w = w_h[:, :]
    bias16 = bias_h[:, :]
    y16 = y_h[:, :]

    with tile.TileContext(nc) as tc:
        with (
            tc.tile_pool(name="consts", bufs=1) as consts,
            tc.tile_pool(name="persist", bufs=1) as persist,
            tc.tile_pool(name="wpool", bufs=3) as wpool,
            tc.tile_pool(name="wspool", bufs=3) as wspool,
            tc.tile_pool(name="xpool", bufs=3) as xpool,
            tc.tile_pool(name="stats", bufs=4) as stats,
            tc.tile_pool(name="trash", bufs=2) as trash,
            tc.tile_pool(name="ypool", bufs=3) as ypool,
            tc.tile_pool(name="psum", bufs=2, space="PSUM") as psum,
        ):
            eps_t = consts.tile([P, 1], F32)
            nc.vector.memset(eps_t, EPS)

            # persistent operands
            wsT = persist.tile([P, KT, OUT_F], F16)    # sign(w-mu)^T  [i, o]
            xT = persist.tile([P, KT, B], F16)         # x^T -> r*x^T in place
            corr_lhsT = persist.tile([2, B], F16)      # rows: -mu*r, ones
            corr_rhs = persist.tile([2, OUT_F], F16)   # rows: S, bias/beta
            r_bcast = persist.tile([P, B], F16)
            beta_bcast = persist.tile([P, OUT_F], F16)
            stats_cols = persist.tile([P, P], F16)     # cols 0..7 -mu*r, 8..15 r
            statsT = persist.tile([P, P], F16)
            # cols 0..15 S, 16..31 beta, 32..47 bias/beta per o-tile
            s_cols = persist.tile([P, P], F16)
            sT = persist.tile([P, P], F16)
            bias_cols = persist.tile([P, OT], F16)

            nc.vector.memset(stats_cols, 0.0)
            nc.vector.memset(s_cols, 0.0)
            # ones row for the corr matmul; engine ops can't target partition
            # 1 directly, so memset at partition 0 and DMA into place
            ones_row = consts.tile([1, B], F16)
            nc.vector.memset(ones_row, 1.0)
            nc.sync.dma_start(out=corr_lhsT[1:2, :], in_=ones_row)

            # bias in column layout [128, OT] (transposed load)
            bias_tiles = bias16.rearrange("a (t c) -> (a t) c", c=P)  # [16,128]
            nc.sync.dma_start(out=bias_cols, in_=bias_tiles, transpose=True)

            # ---------------- weight pipeline ----------------
            for ot in range(OT):
                wt = wpool.tile([P, IN_F], F32)
                nc.gpsimd.dma_start(out=wt, in_=w[ot * P:(ot + 1) * P, :])

                sumw = stats.tile([P, 1], F32)
                nc.vector.reduce_sum(out=sumw, in_=wt, axis=AXF)
                negmean = stats.tile([P, 1], F32)
                nc.vector.tensor_scalar_mul(
                    out=negmean, in0=sumw, scalar1=-1.0 / IN_F)

                tr = trash.tile([P, IN_F], F16)
                asum = stats.tile([P, 1], F32)
                nc.scalar.activation(
                    out=tr, in_=wt, func=AF.Abs, accum_out=asum)
                beta = stats.tile([P, 1], F32)
                nc.vector.tensor_scalar_mul(
                    out=beta, in0=asum, scalar1=1.0 / IN_F)

                wb = wspool.tile([P, IN_F], F16)
                ssum = stats.tile([P, 1], F32)
                nc.scalar.activation(
                    out=wb, in_=wt, func=AF.Sign, bias=negmean, scale=1.0,
                    accum_out=ssum)

                # S, beta, bias/beta columns for this o-tile
                nc.vector.tensor_copy(out=s_cols[:, ot:ot + 1], in_=ssum)
                nc.vector.tensor_copy(
                    out=s_cols[:, 16 + ot:16 + ot + 1], in_=beta)
                binv = stats.tile([P, 1], F32)
                nc.vector.reciprocal(out=binv, in_=beta)
                nc.vector.tensor_mul(
                    out=s_cols[:, 32 + ot:32 + ot + 1],
                    in0=bias_cols[:, ot:ot + 1], in1=binv)

                # one 3D-out xbar transpose per o-tile: logical [2048, 128]
                # transpose lands row i at (p=i//KT, k=i%KT). Alternate the
                # issuing HWDGE engine so transpose issue latency (~2.5us
                # each) doesn't serialize on one engine queue.
                teng = nc.sync if ot % 2 == 0 else nc.scalar
                teng.dma_start(
                    out=wsT[:, :, ot * P:(ot + 1) * P], in_=wb,
                    transpose=True)

            # S/beta/bias' -> row layout [1, OUT_F] via xbar + DRAM bounce
            nc.sync.dma_start(out=sT, in_=s_cols, transpose=True)
            s_rows = s_h[0:1, :].rearrange("a (b c) -> (a b) c", c=P)
            beta_rows = beta_h[0:1, :].rearrange("a (b c) -> (a b) c", c=P)
            bpr_rows = bpr_h[0:1, :].rearrange("a (b c) -> (a b) c", c=P)
            nc.sync.dma_start(out=s_rows, in_=sT[0:OT, :])
            nc.sync.dma_start(out=beta_rows, in_=sT[16:16 + OT, :])
            nc.sync.dma_start(out=bpr_rows, in_=sT[32:32 + OT, :])
            nc.sync.dma_start(out=corr_rhs[0:1, :], in_=s_h[0:1, :])
            nc.sync.dma_start(out=corr_rhs[1:2, :], in_=bpr_h[0:1, :])
            b_src = beta_h[0:1, :]
            b_bc_ap = bass.AP(
                tensor=b_src.tensor, offset=b_src.offset,
                ap=[[0, P], [1, OUT_F]])
            nc.gpsimd.dma_start(out=beta_bcast, in_=b_bc_ap)

            # ---------------- x pipeline ----------------
            for bt in range(BT):
                xt = xpool.tile([P, IN_F], F16)
                nc.gpsimd.dma_start(out=xt, in_=x16[bt * P:(bt + 1) * P, :])
                st = stats.tile([P, 4, 6], F32)
                for g in range(4):
                    nc.vector.bn_stats(
                        out=st[:, g, :], in_=xt[:, g * 512:(g + 1) * 512])
                mv = stats.tile([P, 2], F32)
                nc.vector.bn_aggr(out=mv, in_=st)
                std = stats.tile([P, 1], F32)
                nc.scalar.activation(
                    out=std, in_=mv[:, 1:2], func=AF.Sqrt, bias=eps_t,
                    scale=1.0)
                r32 = stats.tile([P, 1], F32)
                nc.vector.reciprocal(out=r32, in_=std)
                # -(mu * r) and r columns, fp16
                nc.vector.tensor_scalar(
                    out=stats_cols[:, bt:bt + 1], in0=mv[:, 0:1], scalar1=r32,
                    scalar2=-1.0, op0=MUL, op1=MUL)
                nc.vector.tensor_copy(
                    out=stats_cols[:, 8 + bt:8 + bt + 1], in_=r32)

            # transposed load of x (DRAM -> SBUF via xbar), same 3D-out row
            # interleave as wsT: row i -> (p=i//KT, k=i%KT)
            nc.sync.dma_start(out=xT, in_=x16, transpose=True)

            # stats -> row layout + broadcast
            nc.sync.dma_start(out=statsT, in_=stats_cols, transpose=True)
            nmr_rows = nmr_h[0:1, :].rearrange("a (b c) -> (a b) c", c=P)
            r_rows = r_h[0:1, :].rearrange("a (b c) -> (a b) c", c=P)
            nc.sync.dma_start(out=nmr_rows, in_=statsT[0:BT, :])
            nc.sync.dma_start(out=r_rows, in_=statsT[8:8 + BT, :])
            nc.sync.dma_start(out=corr_lhsT[0:1, :], in_=nmr_h[0:1, :])
            r_src = r_h[0:1, :]
            r_bc_ap = bass.AP(
                tensor=r_src.tensor, offset=r_src.offset,
                ap=[[0, P], [1, B]])
            nc.gpsimd.dma_start(out=r_bcast, in_=r_bc_ap)

            # scale x^T by r (in place)
            for k in range(KT):
                nc.vector.tensor_mul(
                    out=xT[:, k, :], in0=xT[:, k, :], in1=r_bcast)

            # ---------------- matmul + epilogue ----------------
            for bt in range(BT):
                ps = psum.tile([P, OUT_F], F32)
                bsl = slice(bt * P, (bt + 1) * P)
                # corr matmul LAST so the main matmuls can start before the
                # full weight-stats pipeline (S, bias/beta rows) finishes
                for k in range(KT):
                    for oc in range(OC):
                        osl = slice(oc * NC_CHUNK, (oc + 1) * NC_CHUNK)
                        nc.tensor.matmul(
                            ps[:, osl], xT[:, k, bsl], wsT[:, k, osl],
                            start=(k == 0), stop=False)
                for oc in range(OC):
                    osl = slice(oc * NC_CHUNK, (oc + 1) * NC_CHUNK)
                    nc.tensor.matmul(
                        ps[:, osl], corr_lhsT[:, bsl], corr_rhs[:, osl],
                        start=False, stop=True)
                ysb = ypool.tile([P, OUT_F], F16)
                nc.vector.tensor_mul(out=ysb, in0=ps, in1=beta_bcast)
                nc.gpsimd.dma_start(out=y16[bt * P:(bt + 1) * P, :], in_=ysb)

    return nc


_NC_CACHE = None
LAST_RESULT = None


def _get_program():
    global _NC_CACHE
    if _NC_CACHE is None:
        nc = _build_program()
        nc.finalize()
        _NC_CACHE = nc
    return _NC_CACHE


def kernel(x: np.ndarray, weight: np.ndarray, bias: np.ndarray) -> np.ndarray:
    global LAST_RESULT
    assert x.shape == (BATCH, IN_F) and weight.shape == (OUT_F, IN_F)

    nc = _get_program()

    x16 = np.ascontiguousarray(x.astype(np.float16))
    w32 = np.ascontiguousarray(weight.astype(np.float32))
    b16 = np.ascontiguousarray(bias.astype(np.float16).reshape(1, OUT_F))

    in_maps = []
    for c in range(N_CORES):
        in_maps.append({
            "x16": np.ascontiguousarray(x16[c * B:(c + 1) * B, :]),
            "w": w32,
            "bias16": b16,
        })

    trace = bool(int(os.environ.get("BITLIN_TRACE", "0")))
    res = run_bass_kernel_spmd(
        nc, in_maps, core_ids=list(range(N_CORES)), trace=trace)
    LAST_RESULT = res

    y = np.concatenate(
        [np.asarray(res.results[c]["y16"]) for c in range(N_CORES)], axis=0)
    return y.astype(np.float32)
